# revision 34
# baseline (speedup 1.0000x reference)
"""DIAMSoftmaxLoss on 8 Trainium2 NeuronCores.

Strategy (classification/tensor parallel over the class axis):
  - Host: fn = l2norm(input) (tiny), shard weight rows 8 ways
    (12500/core, zero-padded to 12544 whole 128-row tiles).
  - Device k (SPMD, same program, different shard), software-pipelined
    in 9 class-groups (512 leader + 8x1536) with prep two groups ahead:
      * prep(g): DMA the group's weight rows into SBUF; row sumsq via
        DVE scalar_tensor_tensor with fused accum; 1/sqrt as ACT
        Ln + Exp(-0.5*x) (same activation-table set as the softmax Exp,
        avoiding ~2.7us table-set thrash); scale rows to bf16;
        PE-transpose into a resident wnT_g [128d x group] via a 2-bank
        PSUM staging tile; stream the raw rows back out to w_out (the
        memory-heavy base copy for the weight update) on the SWDGE
        queue so stores never block loads.
      * mm_exp(g, bi): logits = fnT_bi.T @ wnT_g into a 3-bank PSUM
        group [128b x 1536c] (2 bufs ping-pong; ACT is the bottleneck
        engine and everything else hides under it); one ACT pass
        Exp(30*x) with fused accum_out yields the per-row partial sums
        of exp(S*cos) directly -- no separate reduce.
  - Host: global sum over cores/groups, subtract the pad contribution
    (pad rows give exp(0)=1 each) and the label term, then
    loss = mean(softplus(M + log(sum_neg) - S*pos)).
    Weight update: per-class mean of fn for present classes patched
    over the device-copied weight (O(batch) rows).
"""

import os

import numpy as np
import ml_dtypes

BS, IN_F, OUT_F = 1024, 128, 100000
MARGIN, SCALE = 0.35, 30.0
N_CORES = 8
C_PER = OUT_F // N_CORES          # 12500 real classes per core
C_PAD = 12544                     # zero-padded to whole 128-row tiles
CHUNK = 2048                      # transpose staging width (2 PSUM banks bf16)
N_PAD_TOTAL = (C_PAD - C_PER) * N_CORES

# matmul/exp PSUM groups of 1536 (3 banks f32 x 2 bufs; the 2-bank bf16
# transpose staging tile brings PSUM to exactly 8 banks). The prep grid is
# identical to the group grid (1:1), with a small 512 leader so the
# software pipeline fills quickly.
GROUP = 1536
_GROUPS = [(0, 512)]
_off = 512
while _off < C_PAD:
    _GROUPS.append((_off, min(GROUP, C_PAD - _off)))
    _off += GROUP
N_GROUPS = len(_GROUPS)           # 9: 512 + 8x1536
_CHUNKS = _GROUPS
N_CHUNKS = N_GROUPS

_NC_CACHE = None
LAST_EXEC_NS = None


def _split_sync_waits(nc, max_waits=1):
    """This container's walrus supports only `max_waits` sync-wait commands
    per ISA instruction; hoist extras onto inserted same-engine NoOps."""
    from concourse import mybir

    for func in nc.m.functions:
        for blk in func.blocks:
            insts = blk.instructions
            i = 0
            while i < len(insts):
                ins = insts[i]
                si = ins.sync_info
                if si is not None and len(si.on_wait) > max_waits:
                    waits = list(si.on_wait)
                    extra, keep = waits[:-max_waits], waits[-max_waits:]
                    pos = i
                    for j in range(0, len(extra), max_waits):
                        nop = mybir.InstNoOp(
                            name=nc.get_next_instruction_name(), ins=[], outs=[]
                        )
                        nop.engine = ins.engine
                        nop.sync_info = mybir.SyncInfo(
                            on_wait=extra[j : j + max_waits], on_update=[]
                        )
                        nc.register_instruction(nop)
                        insts.insert(pos, nop)
                        pos += 1
                        i += 1
                    si.on_wait = keep
                i += 1


def _build_bass(repeat=1):
    import concourse.bass as bass
    import concourse.tile as tile
    from concourse import mybir

    nc = bass.Bass("TRN2", debug=False, num_devices=N_CORES)
    f32 = mybir.dt.float32
    bf16 = mybir.dt.bfloat16

    w = nc.dram_tensor("w", [C_PAD, IN_F], f32, kind="ExternalInput").ap()
    fnT = nc.dram_tensor("fnT", [IN_F, BS], bf16, kind="ExternalInput").ap()
    ident = nc.dram_tensor("ident", [128, 128], bf16, kind="ExternalInput").ap()
    w_out = nc.dram_tensor("w_out", [C_PAD, IN_F], f32, kind="ExternalOutput").ap()
    partials = nc.dram_tensor(
        "partials", [128, N_GROUPS * 8], f32, kind="ExternalOutput"
    ).ap()

    # view class rows as (tile, partition) x d
    w_r = w.rearrange("(t p) d -> p t d", p=128)        # [128, 100, 128]
    wo_r = w_out.rearrange("(t p) d -> p t d", p=128)

    n_tiles_total = C_PAD // 128                        # 100

    with tile.TileContext(nc) as tc:
        with (
            tc.tile_pool(name="persist", bufs=1) as persist,
            tc.tile_pool(name="wchunk", bufs=3) as wpool,
            tc.tile_pool(name="scratch", bufs=4) as scratch,
            tc.tile_pool(name="psum_mm", bufs=2, space="PSUM") as psum_mm,
            tc.tile_pool(name="psum_tp", bufs=1, space="PSUM") as psum_tp,
        ):
            FnT = persist.tile([128, BS], bf16)
            Ident = persist.tile([128, 128], bf16)
            # one wnT tile per group: avoids false whole-tile deps between
            # prep(g+2) writes and group g's matmul reads
            WnTg = [
                persist.tile([128, _GROUPS[g][1]], bf16, name=f"wnt{g}")
                for g in range(N_GROUPS)
            ]
            SS = persist.tile([128, n_tiles_total], f32)
            INV = persist.tile([128, n_tiles_total], f32)
            PART = persist.tile([128, N_GROUPS * 8], f32)

            def prep(g, after_load=None):
                """Load chunk g, compute row 1/norm, PE-transpose normalized
                bf16 rows into WnT columns; stream raw rows out."""
                c0, csz = _CHUNKS[g]
                nt = csz // 128
                t0 = c0 // 128
                Wg = wpool.tile([128, 16, 128], f32, tag="wg", name=f"wg{g}")
                nc.sync.dma_start(out=Wg[:, :nt, :], in_=w_r[:, t0 : t0 + nt, :])
                if after_load is not None:
                    after_load()
                for t in range(nt):
                    sq = scratch.tile([128, 128], f32, tag="sq", name=f"sq{g}_{t}")
                    nc.vector.scalar_tensor_tensor(
                        out=sq,
                        in0=Wg[:, t, :],
                        scalar=1.0,
                        in1=Wg[:, t, :],
                        op0=mybir.AluOpType.mult,
                        op1=mybir.AluOpType.mult,
                        accum_out=SS[:, t0 + t : t0 + t + 1],
                    )
                nc.vector.tensor_scalar_max(
                    out=SS[:, t0 : t0 + nt], in0=SS[:, t0 : t0 + nt], scalar1=1e-24
                )
                # inv = exp(-0.5 * ln(ss)) = 1/sqrt(ss); Ln+Exp share one
                # activation-table set with the softmax Exp below.
                nc.scalar.activation(
                    out=SS[:, t0 : t0 + nt],
                    in_=SS[:, t0 : t0 + nt],
                    func=mybir.ActivationFunctionType.Ln,
                )
                nc.scalar.activation(
                    out=INV[:, t0 : t0 + nt],
                    in_=SS[:, t0 : t0 + nt],
                    func=mybir.ActivationFunctionType.Exp,
                    scale=-0.5,
                )
                TP = psum_tp.tile([128, CHUNK], bf16, tag="tp", name=f"tp{g}")
                for t in range(nt):
                    Wn = scratch.tile([128, 128], bf16, tag="wn", name=f"wn{g}_{t}")
                    nc.vector.tensor_scalar_mul(
                        out=Wn,
                        in0=Wg[:, t, :],
                        scalar1=INV[:, t0 + t : t0 + t + 1],
                    )
                    nc.tensor.transpose(
                        out=TP[:, t * 128 : (t + 1) * 128], in_=Wn, identity=Ident
                    )
                nc.vector.tensor_copy(out=WnTg[g], in_=TP[:, :csz])
                # stream the raw shard back out (weight-update base copy) on
                # the SWDGE (gpsimd) queue so stores never block loads
                nc.gpsimd.dma_start(out=wo_r[:, t0 : t0 + nt, :], in_=Wg[:, :nt, :])

            def mm_exp(gi, bi):
                """logits + fused exp/row-sum for one psum group x b-tile."""
                q0, qsz = _GROUPS[gi]
                PS = psum_mm.tile([128, GROUP], f32, tag="ps", name=f"ps{gi}_{bi}")
                for off in range(0, qsz, 512):
                    n = min(512, qsz - off)
                    nc.tensor.matmul(
                        PS[:, off : off + n],
                        lhsT=FnT[:, bi * 128 : (bi + 1) * 128],
                        rhs=WnTg[gi][:, off : off + n],
                        start=True,
                        stop=True,
                    )
                nc.scalar.activation(
                    out=PS[:, :qsz],
                    in_=PS[:, :qsz],
                    func=mybir.ActivationFunctionType.Exp,
                    scale=SCALE,
                    accum_out=PART[:, gi * 8 + bi : gi * 8 + bi + 1],
                )

            for _rep in range(repeat):
                # software pipeline: prep runs two groups ahead of mm/exp so
                # the (in-order) ACT queue sees ln/inv well before the
                # matching group's exps and DVE/PE prep hides under them.
                prep(
                    0,
                    after_load=lambda: (
                        nc.sync.dma_start(out=FnT, in_=fnT),
                        nc.sync.dma_start(out=Ident, in_=ident),
                    ),
                )
                prep(1)
                for gi in range(N_GROUPS):
                    if gi + 2 < N_GROUPS:
                        prep(gi + 2)
                    for bi in range(8):
                        mm_exp(gi, bi)
                    # stream this group's partial sums out incrementally on
                    # the ACT HWDGE queue (wait already satisfied there)
                    nc.scalar.dma_start(
                        out=partials[:, gi * 8 : (gi + 1) * 8],
                        in_=PART[:, gi * 8 : (gi + 1) * 8],
                    )
    _split_sync_waits(nc)
    return nc


def _get_nc():
    global _NC_CACHE
    if _NC_CACHE is None:
        _NC_CACHE = _build_bass()
    return _NC_CACHE


def _l2norm_rows(x):
    n = np.sqrt((x * x).sum(axis=-1, keepdims=True))
    return x / np.maximum(n, 1e-12)


def kernel(input, label, weight):
    global LAST_EXEC_NS
    from concourse import bass_utils

    input = np.ascontiguousarray(np.asarray(input), dtype=np.float32)
    label = np.asarray(label).astype(np.int64)
    weight = np.ascontiguousarray(np.asarray(weight), dtype=np.float32)

    fn32 = _l2norm_rows(input).astype(np.float32)
    fnT_bf = np.ascontiguousarray(fn32.T).astype(ml_dtypes.bfloat16)
    ident = np.eye(128, dtype=ml_dtypes.bfloat16)

    in_maps = []
    for k in range(N_CORES):
        wsh = np.zeros((C_PAD, IN_F), np.float32)
        wsh[:C_PER] = weight[k * C_PER : (k + 1) * C_PER]
        in_maps.append({"w": wsh, "fnT": fnT_bf, "ident": ident})

    nc = _get_nc()
    trace = bool(int(os.environ.get("KERNEL_TRACE", "0")))
    try:
        import antenv.axon_hooks  # noqa: F401
    except ImportError:
        # No NTFF hook in this container: force-disable tracing so an
        # externally-set BASS_TRACE can't crash the axon execute path.
        os.environ["BASS_NEVER_TRACE"] = "1"
        trace = False
    res = bass_utils.run_bass_kernel_spmd(
        nc, in_maps, core_ids=list(range(N_CORES)), trace=trace
    )
    LAST_EXEC_NS = res.exec_time_ns
    outs = res.results

    new_w = np.concatenate(
        [outs[k]["w_out"][:C_PER] for k in range(N_CORES)], axis=0
    )

    # global per-row sum of exp(S * cos) over all (padded) classes
    part = np.stack([outs[k]["partials"] for k in range(N_CORES)])  # [8,128,72]
    sums = part.astype(np.float64).sum(axis=0)                      # [128, 72]
    sums = sums.reshape(128, N_GROUPS, 8).sum(axis=1)               # [128, bi]
    s_all = sums.T.reshape(BS)                                      # b = bi*128+p

    # label (positive) term, computed on host in fp64
    wl = weight[label].astype(np.float64)
    wn_l = wl / np.maximum(np.sqrt((wl * wl).sum(-1, keepdims=True)), 1e-12)
    pos = SCALE * (fn32.astype(np.float64) * wn_l).sum(-1)

    sum_neg = s_all - float(N_PAD_TOTAL) - np.exp(pos)
    sum_neg = np.maximum(sum_neg, 1e-300)
    z = MARGIN + np.log(sum_neg) - pos
    # stable softplus
    loss = np.mean(np.log1p(np.exp(-np.abs(z))) + np.maximum(z, 0.0))

    # segment-mean weight update for classes present in the batch
    uniq, inv_idx, counts = np.unique(
        label, return_inverse=True, return_counts=True
    )
    acc = np.zeros((len(uniq), IN_F), np.float64)
    np.add.at(acc, inv_idx, fn32.astype(np.float64))
    new_w[uniq] = (acc / counts[:, None]).astype(np.float32)

    return np.float32(loss), new_w


# revision 36
# speedup vs baseline: 1.0103x; 1.0103x over previous
"""DIAMSoftmaxLoss on 8 Trainium2 NeuronCores.

Strategy (classification/tensor parallel over the class axis):
  - Host: fn = l2norm(input) (tiny), shard weight rows 8 ways
    (12500/core, zero-padded to 12544 whole 128-row tiles).
  - Device k (SPMD, same program, different shard), software-pipelined
    in 9 class-groups (512 leader + 8x1536) with prep three groups ahead:
      * prep(g): DMA the group's weight rows into SBUF; row sumsq via
        DVE scalar_tensor_tensor with fused accum; 1/sqrt as ACT
        Ln + Exp(-0.5*x) (same activation-table set as the softmax Exp,
        avoiding ~2.7us table-set thrash); scale rows to bf16;
        PE-transpose into a resident wnT_g [128d x group] via a 2-bank
        PSUM staging tile; stream the raw rows back out to w_out (the
        memory-heavy base copy for the weight update) on the SWDGE
        queue so stores never block loads.
      * mm_exp(g, bi): logits = fnT_bi.T @ wnT_g into a 3-bank PSUM
        group [128b x 1536c] (2 bufs ping-pong; ACT is the bottleneck
        engine and everything else hides under it); one ACT pass
        Exp(30*x) with fused accum_out yields the per-row partial sums
        of exp(S*cos) directly -- no separate reduce.
  - Host: global sum over cores/groups, subtract the pad contribution
    (pad rows give exp(0)=1 each) and the label term, then
    loss = mean(softplus(M + log(sum_neg) - S*pos)).
    Weight update: per-class mean of fn for present classes patched
    over the device-copied weight (O(batch) rows).
"""

import os

import numpy as np
import ml_dtypes

BS, IN_F, OUT_F = 1024, 128, 100000
MARGIN, SCALE = 0.35, 30.0
N_CORES = 8
C_PER = OUT_F // N_CORES          # 12500 real classes per core
C_PAD = 12544                     # zero-padded to whole 128-row tiles
CHUNK = 2048                      # transpose staging width (2 PSUM banks bf16)
N_PAD_TOTAL = (C_PAD - C_PER) * N_CORES

# matmul/exp PSUM groups of 1536 (3 banks f32 x 2 bufs; the 2-bank bf16
# transpose staging tile brings PSUM to exactly 8 banks). The prep grid is
# identical to the group grid (1:1), with a small 512 leader so the
# software pipeline fills quickly.
GROUP = 1536
_GROUPS = [(0, 512)]
_off = 512
while _off < C_PAD:
    _GROUPS.append((_off, min(GROUP, C_PAD - _off)))
    _off += GROUP
N_GROUPS = len(_GROUPS)           # 9: 512 + 8x1536
_CHUNKS = _GROUPS
N_CHUNKS = N_GROUPS

_NC_CACHE = None
LAST_EXEC_NS = None


def _split_sync_waits(nc, max_waits=1):
    """This container's walrus supports only `max_waits` sync-wait commands
    per ISA instruction; hoist extras onto inserted same-engine NoOps."""
    from concourse import mybir

    for func in nc.m.functions:
        for blk in func.blocks:
            insts = blk.instructions
            i = 0
            while i < len(insts):
                ins = insts[i]
                si = ins.sync_info
                if si is not None and len(si.on_wait) > max_waits:
                    waits = list(si.on_wait)
                    extra, keep = waits[:-max_waits], waits[-max_waits:]
                    pos = i
                    for j in range(0, len(extra), max_waits):
                        nop = mybir.InstNoOp(
                            name=nc.get_next_instruction_name(), ins=[], outs=[]
                        )
                        nop.engine = ins.engine
                        nop.sync_info = mybir.SyncInfo(
                            on_wait=extra[j : j + max_waits], on_update=[]
                        )
                        nc.register_instruction(nop)
                        insts.insert(pos, nop)
                        pos += 1
                        i += 1
                    si.on_wait = keep
                i += 1


def _build_bass(repeat=1):
    import concourse.bass as bass
    import concourse.tile as tile
    from concourse import mybir

    nc = bass.Bass("TRN2", debug=False, num_devices=N_CORES)
    f32 = mybir.dt.float32
    bf16 = mybir.dt.bfloat16

    w = nc.dram_tensor("w", [C_PAD, IN_F], f32, kind="ExternalInput").ap()
    fnT = nc.dram_tensor("fnT", [IN_F, BS], bf16, kind="ExternalInput").ap()
    ident = nc.dram_tensor("ident", [128, 128], bf16, kind="ExternalInput").ap()
    w_out = nc.dram_tensor("w_out", [C_PAD, IN_F], f32, kind="ExternalOutput").ap()
    partials = nc.dram_tensor(
        "partials", [128, N_GROUPS * 8], f32, kind="ExternalOutput"
    ).ap()

    # view class rows as (tile, partition) x d
    w_r = w.rearrange("(t p) d -> p t d", p=128)        # [128, 100, 128]
    wo_r = w_out.rearrange("(t p) d -> p t d", p=128)

    n_tiles_total = C_PAD // 128                        # 100

    with tile.TileContext(nc) as tc:
        with (
            tc.tile_pool(name="persist", bufs=1) as persist,
            tc.tile_pool(name="wchunk", bufs=4) as wpool,
            tc.tile_pool(name="scratch", bufs=4) as scratch,
            tc.tile_pool(name="psum_mm", bufs=2, space="PSUM") as psum_mm,
            tc.tile_pool(name="psum_tp", bufs=1, space="PSUM") as psum_tp,
        ):
            FnT = persist.tile([128, BS], bf16)
            Ident = persist.tile([128, 128], bf16)
            # one wnT tile per group: avoids false whole-tile deps between
            # prep(g+2) writes and group g's matmul reads
            WnTg = [
                persist.tile([128, _GROUPS[g][1]], bf16, name=f"wnt{g}")
                for g in range(N_GROUPS)
            ]
            SS = persist.tile([128, n_tiles_total], f32)
            INV = persist.tile([128, n_tiles_total], f32)
            PART = persist.tile([128, N_GROUPS * 8], f32)

            def prep(g, after_load=None):
                """Load chunk g, compute row 1/norm, PE-transpose normalized
                bf16 rows into WnT columns; stream raw rows out."""
                c0, csz = _CHUNKS[g]
                nt = csz // 128
                t0 = c0 // 128
                Wg = wpool.tile([128, 16, 128], f32, tag="wg", name=f"wg{g}")
                nc.sync.dma_start(out=Wg[:, :nt, :], in_=w_r[:, t0 : t0 + nt, :])
                if after_load is not None:
                    after_load()
                for t in range(nt):
                    sq = scratch.tile([128, 128], f32, tag="sq", name=f"sq{g}_{t}")
                    nc.vector.scalar_tensor_tensor(
                        out=sq,
                        in0=Wg[:, t, :],
                        scalar=1.0,
                        in1=Wg[:, t, :],
                        op0=mybir.AluOpType.mult,
                        op1=mybir.AluOpType.mult,
                        accum_out=SS[:, t0 + t : t0 + t + 1],
                    )
                nc.vector.tensor_scalar_max(
                    out=SS[:, t0 : t0 + nt], in0=SS[:, t0 : t0 + nt], scalar1=1e-24
                )
                # inv = exp(-0.5 * ln(ss)) = 1/sqrt(ss); Ln+Exp share one
                # activation-table set with the softmax Exp below.
                nc.scalar.activation(
                    out=SS[:, t0 : t0 + nt],
                    in_=SS[:, t0 : t0 + nt],
                    func=mybir.ActivationFunctionType.Ln,
                )
                nc.scalar.activation(
                    out=INV[:, t0 : t0 + nt],
                    in_=SS[:, t0 : t0 + nt],
                    func=mybir.ActivationFunctionType.Exp,
                    scale=-0.5,
                )
                TP = psum_tp.tile([128, CHUNK], bf16, tag="tp", name=f"tp{g}")
                for t in range(nt):
                    Wn = scratch.tile([128, 128], bf16, tag="wn", name=f"wn{g}_{t}")
                    nc.vector.tensor_scalar_mul(
                        out=Wn,
                        in0=Wg[:, t, :],
                        scalar1=INV[:, t0 + t : t0 + t + 1],
                    )
                    nc.tensor.transpose(
                        out=TP[:, t * 128 : (t + 1) * 128], in_=Wn, identity=Ident
                    )
                nc.vector.tensor_copy(out=WnTg[g], in_=TP[:, :csz])
                # stream the raw shard back out (weight-update base copy) on
                # the SWDGE (gpsimd) queue so stores never block loads
                nc.gpsimd.dma_start(out=wo_r[:, t0 : t0 + nt, :], in_=Wg[:, :nt, :])

            def mm_exp(gi, bi):
                """logits + fused exp/row-sum for one psum group x b-tile."""
                q0, qsz = _GROUPS[gi]
                PS = psum_mm.tile([128, GROUP], f32, tag="ps", name=f"ps{gi}_{bi}")
                for off in range(0, qsz, 512):
                    n = min(512, qsz - off)
                    nc.tensor.matmul(
                        PS[:, off : off + n],
                        lhsT=FnT[:, bi * 128 : (bi + 1) * 128],
                        rhs=WnTg[gi][:, off : off + n],
                        start=True,
                        stop=True,
                    )
                nc.scalar.activation(
                    out=PS[:, :qsz],
                    in_=PS[:, :qsz],
                    func=mybir.ActivationFunctionType.Exp,
                    scale=SCALE,
                    accum_out=PART[:, gi * 8 + bi : gi * 8 + bi + 1],
                )

            for _rep in range(repeat):
                # software pipeline: prep runs two groups ahead of mm/exp so
                # the (in-order) ACT queue sees ln/inv well before the
                # matching group's exps and DVE/PE prep hides under them.
                prep(
                    0,
                    after_load=lambda: (
                        nc.sync.dma_start(out=FnT, in_=fnT),
                        nc.sync.dma_start(out=Ident, in_=ident),
                    ),
                )
                prep(1)
                prep(2)
                for gi in range(N_GROUPS):
                    if gi + 3 < N_GROUPS:
                        prep(gi + 3)
                    for bi in range(8):
                        mm_exp(gi, bi)
                    # stream this group's partial sums out incrementally on
                    # the ACT HWDGE queue (wait already satisfied there)
                    nc.scalar.dma_start(
                        out=partials[:, gi * 8 : (gi + 1) * 8],
                        in_=PART[:, gi * 8 : (gi + 1) * 8],
                    )
    _split_sync_waits(nc)
    return nc


def _get_nc():
    global _NC_CACHE
    if _NC_CACHE is None:
        _NC_CACHE = _build_bass()
    return _NC_CACHE


def _l2norm_rows(x):
    n = np.sqrt((x * x).sum(axis=-1, keepdims=True))
    return x / np.maximum(n, 1e-12)


def kernel(input, label, weight):
    global LAST_EXEC_NS
    from concourse import bass_utils

    input = np.ascontiguousarray(np.asarray(input), dtype=np.float32)
    label = np.asarray(label).astype(np.int64)
    weight = np.ascontiguousarray(np.asarray(weight), dtype=np.float32)

    fn32 = _l2norm_rows(input).astype(np.float32)
    fnT_bf = np.ascontiguousarray(fn32.T).astype(ml_dtypes.bfloat16)
    ident = np.eye(128, dtype=ml_dtypes.bfloat16)

    in_maps = []
    for k in range(N_CORES):
        wsh = np.zeros((C_PAD, IN_F), np.float32)
        wsh[:C_PER] = weight[k * C_PER : (k + 1) * C_PER]
        in_maps.append({"w": wsh, "fnT": fnT_bf, "ident": ident})

    nc = _get_nc()
    trace = bool(int(os.environ.get("KERNEL_TRACE", "0")))
    try:
        import antenv.axon_hooks  # noqa: F401
    except ImportError:
        # No NTFF hook in this container: force-disable tracing so an
        # externally-set BASS_TRACE can't crash the axon execute path.
        os.environ["BASS_NEVER_TRACE"] = "1"
        trace = False
    res = bass_utils.run_bass_kernel_spmd(
        nc, in_maps, core_ids=list(range(N_CORES)), trace=trace
    )
    LAST_EXEC_NS = res.exec_time_ns
    outs = res.results

    new_w = np.concatenate(
        [outs[k]["w_out"][:C_PER] for k in range(N_CORES)], axis=0
    )

    # global per-row sum of exp(S * cos) over all (padded) classes
    part = np.stack([outs[k]["partials"] for k in range(N_CORES)])  # [8,128,72]
    sums = part.astype(np.float64).sum(axis=0)                      # [128, 72]
    sums = sums.reshape(128, N_GROUPS, 8).sum(axis=1)               # [128, bi]
    s_all = sums.T.reshape(BS)                                      # b = bi*128+p

    # label (positive) term, computed on host in fp64
    wl = weight[label].astype(np.float64)
    wn_l = wl / np.maximum(np.sqrt((wl * wl).sum(-1, keepdims=True)), 1e-12)
    pos = SCALE * (fn32.astype(np.float64) * wn_l).sum(-1)

    sum_neg = s_all - float(N_PAD_TOTAL) - np.exp(pos)
    sum_neg = np.maximum(sum_neg, 1e-300)
    z = MARGIN + np.log(sum_neg) - pos
    # stable softplus
    loss = np.mean(np.log1p(np.exp(-np.abs(z))) + np.maximum(z, 0.0))

    # segment-mean weight update for classes present in the batch
    uniq, inv_idx, counts = np.unique(
        label, return_inverse=True, return_counts=True
    )
    acc = np.zeros((len(uniq), IN_F), np.float64)
    np.add.at(acc, inv_idx, fn32.astype(np.float64))
    new_w[uniq] = (acc / counts[:, None]).astype(np.float32)

    return np.float32(loss), new_w
